# revision 14
# baseline (speedup 1.0000x reference)
"""Trainium2 Bass kernel for a batch-1 LSTM cell (D=4096).

Math (per reference):
    concat = [h0, x]                       # [1, 8192]
    z  = tanh(concat @ Wc + bc)
    zf = sigmoid(concat @ Wf + bf)
    zi = sigmoid(concat @ Wi + bi)
    zo = sigmoid(concat @ Wo + bo)
    c  = c0 * zf + z * zi
    h  = zo * tanh(c)                      # [1, 4096]  (returned)

Sharding: tensor-parallel over the gate output dim. Core ci owns output
columns [ci*512, (ci+1)*512) of all four gates: it streams its
[8192, 4*512] weight block from HBM, computes the four gate slices, and
the elementwise state update for its 512 lanes. Outputs are concatenated
on the host.

The dominant cost is streaming the weights (512 MiB fp32 total, 64 MiB
per core). The matmul is a matvec, run with the weight tile as the
stationary operand (fp32 moving operand would run at 1/4 rate) and the
activation chunk [128, 1] as the moving operand, accumulating over the
64 K-chunks into PSUM.

Variants:
  f32A: fp32 weights, weights-stationary matvec. Two passes (gates f,i
        then o,c) so each of the 8 concurrent PSUM accumulation series
        owns a whole bank — no cross-series has_written interference.
  f16B: fp16 weights (host-downcast), activation-chunk stationary,
        weight strips moving at 1 col/cycle. Half the HBM bytes.
"""

import numpy as np

import concourse.bass as bass
import concourse.mybir as mybir
import concourse.tile as tile
from concourse.bass_utils import run_bass_kernel_spmd

D = 4096
K = 2 * D          # concat length, 8192
NCORES = 8
SH = D // NCORES   # 512 output columns per core per gate
NKC = K // 128     # 64 K-chunks of 128

VARIANT = "f16B"
STRIPS_PER_DMA = 4
W_BUFS = 8

_F32 = mybir.dt.float32
_F16 = mybir.dt.float16
_AFT = mybir.ActivationFunctionType


def _new_bass():
    return bass.Bass(
        trn_type="TRN2",
        target_bir_lowering=False,
        debug=False,
        num_devices=NCORES,
    )


# Instruction types walrus lowers with multi-wait support (sequencer loops).
_MULTIWAIT_OK = ("InstAllEngineBarrier", "InstNoOp", "InstISA")


def _split_multiwaits(nc):
    """Walrus encodes at most one sync-wait on compute/DMA instructions on
    this toolchain (static-DMA DIRECT2D / S3_LW structs). Tile's semaphore
    assigner sometimes emits 2+. Hoist the extras onto same-engine no-ops
    inserted immediately before the instruction — the sequencer executes the
    nop waits first, which is semantically identical."""
    n = 0
    for fn in nc.m.functions:
        for blk in fn.blocks:
            insts = blk.instructions
            i = 0
            while i < len(insts):
                inst = insts[i]
                si = inst.sync_info
                waits = list(si.on_wait) if si and si.on_wait else []
                if (
                    len(waits) > 1
                    and type(inst).__name__ not in _MULTIWAIT_OK
                ):
                    for w in waits[:-1]:
                        nop = mybir.InstNoOp(
                            name=f"I-waitnop{n}",
                            engine=inst.engine,
                            ins=[],
                            outs=[],
                            sync_info=mybir.SyncInfo(
                                on_wait=[w], on_update=[]
                            ),
                        )
                        insts.insert(i, nop)
                        n += 1
                        i += 1
                    inst.sync_info = mybir.SyncInfo(
                        on_wait=[waits[-1]], on_update=list(si.on_update)
                    )
                i += 1
    return n


def build_f32A():
    """fp32, weights stationary. Two gate-pair passes, 8 PSUM banks each.

    Weight inputs: wa = [K, 2*SH] (gates f,i), wb = [K, 2*SH] (gates o,c).
    xr   = [128, NKC]  xr[p, kc] = concat[128*kc + p]
    bias = [128, 16]   bias[p, 4*g + t] = b_g[shard + 128*t + p], g in f,i,o,c
    c0s  = [128, 4]    c0s[p, t] = c0[shard + 128*t + p]
    out h = [128, 4]   h[p, t] = h_full[shard + 128*t + p]
    """
    nc = _new_bass()
    wa = nc.dram_tensor("wa", [K, 2 * SH], _F32, kind="ExternalInput").ap()
    wb = nc.dram_tensor("wb", [K, 2 * SH], _F32, kind="ExternalInput").ap()
    xr = nc.dram_tensor("xr", [128, NKC], _F32, kind="ExternalInput").ap()
    bias = nc.dram_tensor("bias", [128, 16], _F32, kind="ExternalInput").ap()
    c0s = nc.dram_tensor("c0s", [128, 4], _F32, kind="ExternalInput").ap()
    hout = nc.dram_tensor("h", [128, 4], _F32, kind="ExternalOutput").ap()

    spd = STRIPS_PER_DMA
    n_chunks = NKC // spd
    with tile.TileContext(nc) as tc:
        with (
            tc.tile_pool(name="consts", bufs=1) as cpool,
            tc.tile_pool(name="wpool", bufs=W_BUFS) as wpool,
            tc.tile_pool(name="ppool", bufs=1, space="PSUM") as ppool,
            tc.tile_pool(name="epool", bufs=1) as epool,
        ):
            xr_s = cpool.tile([128, NKC], _F32, name="xr_s")
            nc.sync.dma_start(out=xr_s, in_=xr)
            bias_s = cpool.tile([128, 16], _F32, name="bias_s")
            nc.sync.dma_start(out=bias_s, in_=bias)
            c0_s = cpool.tile([128, 4], _F32, name="c0_s")
            nc.sync.dma_start(out=c0_s, in_=c0s)

            # pre-activations (bias added), laid out [128, 4*g + t]
            pre = epool.tile([128, 16], _F32, name="pre")

            # 8 accumulator banks, shared by both gate-pair phases (reusing
            # the same tiles avoids pool slot-reuse semaphores, which would
            # pile >1 wait onto a matmul — walrus allows exactly one).
            ps = []
            for i in range(8):
                ps.append(ppool.tile([128, 1], _F32, name=f"ps{i}"))

            for ph, wsrc in ((0, wa), (1, wb)):
                # Wait-consumer: walrus matmuls have one sync-wait slot, but
                # the first matmul of a phase would need two (xr-DMA or
                # psum-evacuation wait plus the weight-chunk DMA wait). Run a
                # throwaway complete accumulation group on ps[0] that
                # consumes the non-DMA wait; the real series then re-starts
                # the bank and overwrites.
                nc.tensor.matmul(
                    ps[0][0:1, 0:1],
                    xr_s[:, 0:1],
                    xr_s[:, 0:1],
                    start=True,
                    stop=True,
                )
                for c in range(n_chunks):
                    w = wpool.tile(
                        [128, spd * 2 * SH], _F32, name=f"w{ph}_{c}", tag="w"
                    )
                    src = wsrc[c * spd * 128 : (c + 1) * spd * 128, :].rearrange(
                        "(s p) n -> p s n", p=128
                    )
                    nc.sync.dma_start(
                        out=w.rearrange("p (s n) -> p s n", s=spd), in_=src
                    )
                    for s in range(spd):
                        kc = c * spd + s
                        for gg in range(2):  # gate within pair
                            for t in range(4):
                                nc.tensor.matmul(
                                    ps[4 * gg + t][:, 0:1],
                                    w[
                                        :,
                                        2 * SH * s
                                        + SH * gg
                                        + 128 * t : 2 * SH * s
                                        + SH * gg
                                        + 128 * t
                                        + 128,
                                    ],
                                    xr_s[:, kc : kc + 1],
                                    start=(kc == 0),
                                    stop=(kc == NKC - 1),
                                )
                # evacuate with bias add: gates 2*ph + gg
                for gg in range(2):
                    g = 2 * ph + gg
                    for t in range(4):
                        nc.vector.tensor_add(
                            pre[:, 4 * g + t : 4 * g + t + 1],
                            ps[4 * gg + t][:, 0:1],
                            bias_s[:, 4 * g + t : 4 * g + t + 1],
                        )

            # gate order: f(0:4), i(4:8), o(8:12), c(12:16)
            sig = epool.tile([128, 12], _F32, name="sig")
            nc.scalar.activation(sig, pre[:, 0:12], _AFT.Sigmoid)
            ztl = epool.tile([128, 4], _F32, name="ztl")
            nc.scalar.activation(ztl, pre[:, 12:16], _AFT.Tanh)
            t1 = epool.tile([128, 4], _F32, name="t1")
            nc.vector.tensor_mul(t1, c0_s, sig[:, 0:4])
            t2 = epool.tile([128, 4], _F32, name="t2")
            nc.vector.tensor_mul(t2, ztl, sig[:, 4:8])
            cn = epool.tile([128, 4], _F32, name="cn")
            nc.vector.tensor_add(cn, t1, t2)
            tcn = epool.tile([128, 4], _F32, name="tcn")
            nc.scalar.activation(tcn, cn, _AFT.Tanh)
            hv = epool.tile([128, 4], _F32, name="hv")
            nc.vector.tensor_mul(hv, sig[:, 8:12], tcn)
            nc.sync.dma_start(out=hout, in_=hv)
    return nc


def prep_f32A(x, h0, c0, Wf, bf, Wi, bi, Wc, bc, Wo, bo):
    concat = np.concatenate([h0[0], x[0]]).astype(np.float32)
    xr = np.ascontiguousarray(concat.reshape(NKC, 128).T)
    in_maps = []
    gw = [Wf, Wi, Wo, Wc]
    gb = [bf, bi, bo, bc]
    for ci in range(NCORES):
        lo = ci * SH
        wa = np.ascontiguousarray(
            np.concatenate([W[:, lo : lo + SH] for W in gw[:2]], axis=1)
        )
        wb = np.ascontiguousarray(
            np.concatenate([W[:, lo : lo + SH] for W in gw[2:]], axis=1)
        )
        bias = np.ascontiguousarray(
            np.concatenate(
                [b[lo : lo + SH].reshape(4, 128).T for b in gb], axis=1
            )
        )
        c0s = np.ascontiguousarray(c0[0, lo : lo + SH].reshape(4, 128).T)
        in_maps.append(
            {"wa": wa, "wb": wb, "xr": xr, "bias": bias, "c0s": c0s}
        )
    return in_maps


def post_f32A(results):
    shards = [r["h"].T.reshape(SH) for r in results]
    return np.concatenate(shards)[None, :].astype(np.float32)


def build_f16B():
    """fp16 weights moving, activation chunk stationary. Single pass.

    w4  = [K, 4*SH] fp16, gate order f,i,o,c along columns
    xr  = [128, NKC] fp16 (stationary chunks)
    bias = [1, 4*SH] fp32, c0s = [1, SH] fp32, out h = [1, SH] fp32
    """
    nc = _new_bass()
    w4 = nc.dram_tensor("w4", [K, 4 * SH], _F16, kind="ExternalInput").ap()
    xr = nc.dram_tensor("xr", [128, NKC], _F16, kind="ExternalInput").ap()
    bias = nc.dram_tensor("bias", [1, 4 * SH], _F32, kind="ExternalInput").ap()
    c0s = nc.dram_tensor("c0s", [1, SH], _F32, kind="ExternalInput").ap()
    hout = nc.dram_tensor("h", [1, SH], _F32, kind="ExternalOutput").ap()

    spd = STRIPS_PER_DMA
    n_chunks = NKC // spd
    with tile.TileContext(nc) as tc:
        with (
            tc.tile_pool(name="consts", bufs=1) as cpool,
            tc.tile_pool(name="wpool", bufs=W_BUFS) as wpool,
            tc.tile_pool(name="ppool", bufs=1, space="PSUM") as ppool,
            tc.tile_pool(name="epool", bufs=1) as epool,
        ):
            xr_s = cpool.tile([128, NKC], _F16, name="xr_s")
            nc.sync.dma_start(out=xr_s, in_=xr)
            bias_s = cpool.tile([1, 4 * SH], _F32, name="bias_s")
            nc.sync.dma_start(out=bias_s, in_=bias)
            c0_s = cpool.tile([1, SH], _F32, name="c0_s")
            nc.sync.dma_start(out=c0_s, in_=c0s)

            # one accumulator bank per gate, [1, 512] each on partition 0
            ps = ppool.tile([1, 4 * SH], _F32, name="ps")
            # wait-consumer (see f32A): absorbs the xr-DMA wait so the first
            # real matmul only needs the weight-chunk DMA wait
            nc.tensor.matmul(
                ps[0:1, 0:1], xr_s[:, 0:1], xr_s[:, 0:1], start=True, stop=True
            )

            for c in range(n_chunks):
                w = wpool.tile(
                    [128, spd * 4 * SH], _F16, name=f"w{c}", tag="w"
                )
                src = w4[c * spd * 128 : (c + 1) * spd * 128, :].rearrange(
                    "(s p) n -> p s n", p=128
                )
                nc.sync.dma_start(
                    out=w.rearrange("p (s n) -> p s n", s=spd), in_=src
                )
                for s in range(spd):
                    kc = c * spd + s
                    for g in range(4):
                        nc.tensor.matmul(
                            ps[0:1, SH * g : SH * (g + 1)],
                            xr_s[:, kc : kc + 1],
                            w[:, 4 * SH * s + SH * g : 4 * SH * s + SH * (g + 1)],
                            start=(kc == 0),
                            stop=(kc == NKC - 1),
                        )

            pre = epool.tile([1, 4 * SH], _F32, name="pre")
            nc.vector.tensor_add(pre, ps[0:1, :], bias_s)
            # gate order: f(0:SH), i(SH:2SH), o(2SH:3SH), c(3SH:4SH)
            sig = epool.tile([1, 3 * SH], _F32, name="sig")
            nc.scalar.activation(sig, pre[:, 0 : 3 * SH], _AFT.Sigmoid)
            ztl = epool.tile([1, SH], _F32, name="ztl")
            nc.scalar.activation(ztl, pre[:, 3 * SH : 4 * SH], _AFT.Tanh)
            t1 = epool.tile([1, SH], _F32, name="t1")
            nc.vector.tensor_mul(t1, c0_s, sig[:, 0:SH])
            t2 = epool.tile([1, SH], _F32, name="t2")
            nc.vector.tensor_mul(t2, ztl, sig[:, SH : 2 * SH])
            cn = epool.tile([1, SH], _F32, name="cn")
            nc.vector.tensor_add(cn, t1, t2)
            tcn = epool.tile([1, SH], _F32, name="tcn")
            nc.scalar.activation(tcn, cn, _AFT.Tanh)
            hv = epool.tile([1, SH], _F32, name="hv")
            nc.vector.tensor_mul(hv, sig[:, 2 * SH : 3 * SH], tcn)
            nc.sync.dma_start(out=hout, in_=hv)
    return nc


def prep_f16B(x, h0, c0, Wf, bf, Wi, bi, Wc, bc, Wo, bo):
    concat = np.concatenate([h0[0], x[0]]).astype(np.float32)
    xr = np.ascontiguousarray(concat.reshape(NKC, 128).T).astype(np.float16)
    in_maps = []
    gw = [Wf, Wi, Wo, Wc]
    gb = [bf, bi, bo, bc]
    for ci in range(NCORES):
        lo = ci * SH
        w4 = np.ascontiguousarray(
            np.concatenate([W[:, lo : lo + SH] for W in gw], axis=1)
        ).astype(np.float16)
        bias = np.ascontiguousarray(
            np.concatenate([b[lo : lo + SH] for b in gb])
        ).astype(np.float32)[None, :]
        c0s = np.ascontiguousarray(c0[0, lo : lo + SH]).astype(np.float32)[
            None, :
        ]
        in_maps.append({"w4": w4, "xr": xr, "bias": bias, "c0s": c0s})
    return in_maps


def post_f16B(results):
    shards = [r["h"].reshape(SH) for r in results]
    return np.concatenate(shards)[None, :].astype(np.float32)


_VARIANTS = {
    "f32A": (build_f32A, prep_f32A, post_f32A),
    "f16B": (build_f16B, prep_f16B, post_f16B),
}


def run_variant(variant, inputs, trace=False, **spmd_kwargs):
    build, prep, post = _VARIANTS[variant]
    nc = build()
    # post-scheduling pass for walrus's one-wait-per-instruction limit
    # (CoreSim can't execute the injected nops, so this is HW-path only)
    _split_multiwaits(nc)
    in_maps = prep(**inputs)
    res = run_bass_kernel_spmd(
        nc, in_maps, list(range(NCORES)), trace=trace, **spmd_kwargs
    )
    return post(res.results), res


def kernel(**inputs):
    out, _ = run_variant(VARIANT, inputs)
    return out


# revision 15
# speedup vs baseline: 1.0325x; 1.0325x over previous
"""Trainium2 Bass kernel for a batch-1 LSTM cell (D=4096).

Math (per reference):
    concat = [h0, x]                       # [1, 8192]
    z  = tanh(concat @ Wc + bc)
    zf = sigmoid(concat @ Wf + bf)
    zi = sigmoid(concat @ Wi + bi)
    zo = sigmoid(concat @ Wo + bo)
    c  = c0 * zf + z * zi
    h  = zo * tanh(c)                      # [1, 4096]  (returned)

Sharding: tensor-parallel over the gate output dim. Core ci owns output
columns [ci*512, (ci+1)*512) of all four gates: it streams its
[8192, 4*512] weight block from HBM, computes the four gate slices, and
the elementwise state update for its 512 lanes. Outputs are concatenated
on the host.

The dominant cost is streaming the weights (512 MiB fp32 total, 64 MiB
per core). The matmul is a matvec, run with the weight tile as the
stationary operand (fp32 moving operand would run at 1/4 rate) and the
activation chunk [128, 1] as the moving operand, accumulating over the
64 K-chunks into PSUM.

Variants:
  f32A: fp32 weights, weights-stationary matvec. Two passes (gates f,i
        then o,c) so each of the 8 concurrent PSUM accumulation series
        owns a whole bank — no cross-series has_written interference.
  f16B: fp16 weights (host-downcast), activation-chunk stationary,
        weight strips moving at 1 col/cycle. Half the HBM bytes.
"""

import numpy as np

import concourse.bass as bass
import concourse.mybir as mybir
import concourse.tile as tile
from concourse.bass_utils import run_bass_kernel_spmd

D = 4096
K = 2 * D          # concat length, 8192
NCORES = 8
SH = D // NCORES   # 512 output columns per core per gate
NKC = K // 128     # 64 K-chunks of 128

VARIANT = "f16B"
STRIPS_PER_DMA = 4
W_BUFS = 8

_F32 = mybir.dt.float32
_F16 = mybir.dt.float16
_AFT = mybir.ActivationFunctionType


def _new_bass():
    return bass.Bass(
        trn_type="TRN2",
        target_bir_lowering=False,
        debug=False,
        num_devices=NCORES,
    )


# Instruction types walrus lowers with multi-wait support (sequencer loops).
_MULTIWAIT_OK = ("InstAllEngineBarrier", "InstNoOp", "InstISA")


def _split_multiwaits(nc):
    """Walrus encodes at most one sync-wait on compute/DMA instructions on
    this toolchain (static-DMA DIRECT2D / S3_LW structs). Tile's semaphore
    assigner sometimes emits 2+. Hoist the extras onto same-engine no-ops
    inserted immediately before the instruction — the sequencer executes the
    nop waits first, which is semantically identical."""
    n = 0
    for fn in nc.m.functions:
        for blk in fn.blocks:
            insts = blk.instructions
            i = 0
            while i < len(insts):
                inst = insts[i]
                si = inst.sync_info
                waits = list(si.on_wait) if si and si.on_wait else []
                if (
                    len(waits) > 1
                    and type(inst).__name__ not in _MULTIWAIT_OK
                ):
                    for w in waits[:-1]:
                        nop = mybir.InstNoOp(
                            name=f"I-waitnop{n}",
                            engine=inst.engine,
                            ins=[],
                            outs=[],
                            sync_info=mybir.SyncInfo(
                                on_wait=[w], on_update=[]
                            ),
                        )
                        insts.insert(i, nop)
                        n += 1
                        i += 1
                    inst.sync_info = mybir.SyncInfo(
                        on_wait=[waits[-1]], on_update=list(si.on_update)
                    )
                i += 1
    return n


def build_f32A():
    """fp32, weights stationary. Two gate-pair passes, 8 PSUM banks each.

    Weight inputs: wa = [K, 2*SH] (gates f,i), wb = [K, 2*SH] (gates o,c).
    xr   = [128, NKC]  xr[p, kc] = concat[128*kc + p]
    bias = [128, 16]   bias[p, 4*g + t] = b_g[shard + 128*t + p], g in f,i,o,c
    c0s  = [128, 4]    c0s[p, t] = c0[shard + 128*t + p]
    out h = [128, 4]   h[p, t] = h_full[shard + 128*t + p]
    """
    nc = _new_bass()
    wa = nc.dram_tensor("wa", [K, 2 * SH], _F32, kind="ExternalInput").ap()
    wb = nc.dram_tensor("wb", [K, 2 * SH], _F32, kind="ExternalInput").ap()
    xr = nc.dram_tensor("xr", [128, NKC], _F32, kind="ExternalInput").ap()
    bias = nc.dram_tensor("bias", [128, 16], _F32, kind="ExternalInput").ap()
    c0s = nc.dram_tensor("c0s", [128, 4], _F32, kind="ExternalInput").ap()
    hout = nc.dram_tensor("h", [128, 4], _F32, kind="ExternalOutput").ap()

    spd = STRIPS_PER_DMA
    n_chunks = NKC // spd
    with tile.TileContext(nc) as tc:
        with (
            tc.tile_pool(name="consts", bufs=1) as cpool,
            tc.tile_pool(name="wpool", bufs=W_BUFS) as wpool,
            tc.tile_pool(name="ppool", bufs=1, space="PSUM") as ppool,
            tc.tile_pool(name="epool", bufs=1) as epool,
        ):
            xr_s = cpool.tile([128, NKC], _F32, name="xr_s")
            nc.sync.dma_start(out=xr_s, in_=xr)
            bias_s = cpool.tile([128, 16], _F32, name="bias_s")
            nc.sync.dma_start(out=bias_s, in_=bias)
            c0_s = cpool.tile([128, 4], _F32, name="c0_s")
            nc.sync.dma_start(out=c0_s, in_=c0s)

            # pre-activations (bias added), laid out [128, 4*g + t]
            pre = epool.tile([128, 16], _F32, name="pre")

            # 8 accumulator banks, shared by both gate-pair phases (reusing
            # the same tiles avoids pool slot-reuse semaphores, which would
            # pile >1 wait onto a matmul — walrus allows exactly one).
            ps = []
            for i in range(8):
                ps.append(ppool.tile([128, 1], _F32, name=f"ps{i}"))

            for ph, wsrc in ((0, wa), (1, wb)):
                # Wait-consumer: walrus matmuls have one sync-wait slot, but
                # the first matmul of a phase would need two (xr-DMA or
                # psum-evacuation wait plus the weight-chunk DMA wait). Run a
                # throwaway complete accumulation group on ps[0] that
                # consumes the non-DMA wait; the real series then re-starts
                # the bank and overwrites.
                nc.tensor.matmul(
                    ps[0][0:1, 0:1],
                    xr_s[:, 0:1],
                    xr_s[:, 0:1],
                    start=True,
                    stop=True,
                )
                for c in range(n_chunks):
                    w = wpool.tile(
                        [128, spd * 2 * SH], _F32, name=f"w{ph}_{c}", tag="w"
                    )
                    src = wsrc[c * spd * 128 : (c + 1) * spd * 128, :].rearrange(
                        "(s p) n -> p s n", p=128
                    )
                    nc.sync.dma_start(
                        out=w.rearrange("p (s n) -> p s n", s=spd), in_=src
                    )
                    for s in range(spd):
                        kc = c * spd + s
                        for gg in range(2):  # gate within pair
                            for t in range(4):
                                nc.tensor.matmul(
                                    ps[4 * gg + t][:, 0:1],
                                    w[
                                        :,
                                        2 * SH * s
                                        + SH * gg
                                        + 128 * t : 2 * SH * s
                                        + SH * gg
                                        + 128 * t
                                        + 128,
                                    ],
                                    xr_s[:, kc : kc + 1],
                                    start=(kc == 0),
                                    stop=(kc == NKC - 1),
                                )
                # evacuate with bias add: gates 2*ph + gg
                for gg in range(2):
                    g = 2 * ph + gg
                    for t in range(4):
                        nc.vector.tensor_add(
                            pre[:, 4 * g + t : 4 * g + t + 1],
                            ps[4 * gg + t][:, 0:1],
                            bias_s[:, 4 * g + t : 4 * g + t + 1],
                        )

            # gate order: f(0:4), i(4:8), o(8:12), c(12:16)
            sig = epool.tile([128, 12], _F32, name="sig")
            nc.scalar.activation(sig, pre[:, 0:12], _AFT.Sigmoid)
            ztl = epool.tile([128, 4], _F32, name="ztl")
            nc.scalar.activation(ztl, pre[:, 12:16], _AFT.Tanh)
            t1 = epool.tile([128, 4], _F32, name="t1")
            nc.vector.tensor_mul(t1, c0_s, sig[:, 0:4])
            t2 = epool.tile([128, 4], _F32, name="t2")
            nc.vector.tensor_mul(t2, ztl, sig[:, 4:8])
            cn = epool.tile([128, 4], _F32, name="cn")
            nc.vector.tensor_add(cn, t1, t2)
            tcn = epool.tile([128, 4], _F32, name="tcn")
            nc.scalar.activation(tcn, cn, _AFT.Tanh)
            hv = epool.tile([128, 4], _F32, name="hv")
            nc.vector.tensor_mul(hv, sig[:, 8:12], tcn)
            nc.sync.dma_start(out=hout, in_=hv)
    return nc


def prep_f32A(x, h0, c0, Wf, bf, Wi, bi, Wc, bc, Wo, bo):
    concat = np.concatenate([h0[0], x[0]]).astype(np.float32)
    xr = np.ascontiguousarray(concat.reshape(NKC, 128).T)
    in_maps = []
    gw = [Wf, Wi, Wo, Wc]
    gb = [bf, bi, bo, bc]
    for ci in range(NCORES):
        lo = ci * SH
        wa = np.ascontiguousarray(
            np.concatenate([W[:, lo : lo + SH] for W in gw[:2]], axis=1)
        )
        wb = np.ascontiguousarray(
            np.concatenate([W[:, lo : lo + SH] for W in gw[2:]], axis=1)
        )
        bias = np.ascontiguousarray(
            np.concatenate(
                [b[lo : lo + SH].reshape(4, 128).T for b in gb], axis=1
            )
        )
        c0s = np.ascontiguousarray(c0[0, lo : lo + SH].reshape(4, 128).T)
        in_maps.append(
            {"wa": wa, "wb": wb, "xr": xr, "bias": bias, "c0s": c0s}
        )
    return in_maps


def post_f32A(results):
    shards = [r["h"].T.reshape(SH) for r in results]
    return np.concatenate(shards)[None, :].astype(np.float32)


def build_f16B():
    """fp16 weights moving, activation chunk stationary. Single pass.

    w4  = [K, 4*SH] fp16, gate order f,i,o,c along columns
    xr  = [128, NKC] fp16 (stationary chunks)
    bias = [1, 4*SH] fp32, c0s = [1, SH] fp32, out h = [1, SH] fp32
    """
    nc = _new_bass()
    w4 = nc.dram_tensor("w4", [K, 4 * SH], _F16, kind="ExternalInput").ap()
    xr = nc.dram_tensor("xr", [128, NKC], _F16, kind="ExternalInput").ap()
    bias = nc.dram_tensor("bias", [1, 4 * SH], _F32, kind="ExternalInput").ap()
    c0s = nc.dram_tensor("c0s", [1, SH], _F32, kind="ExternalInput").ap()
    hout = nc.dram_tensor("h", [1, SH], _F32, kind="ExternalOutput").ap()

    spd = STRIPS_PER_DMA
    n_chunks = NKC // spd
    with tile.TileContext(nc) as tc:
        with (
            tc.tile_pool(name="consts", bufs=1) as cpool,
            tc.tile_pool(name="wpool", bufs=W_BUFS) as wpool,
            tc.tile_pool(name="ppool", bufs=1, space="PSUM") as ppool,
            tc.tile_pool(name="epool", bufs=1) as epool,
        ):
            xr_s = cpool.tile([128, NKC], _F16, name="xr_s")
            nc.sync.dma_start(out=xr_s, in_=xr)
            bias_s = cpool.tile([1, 4 * SH], _F32, name="bias_s")
            nc.sync.dma_start(out=bias_s, in_=bias)
            c0_s = cpool.tile([1, SH], _F32, name="c0_s")
            nc.sync.dma_start(out=c0_s, in_=c0s)

            # one accumulator bank per gate, [1, 512] each on partition 0
            ps = ppool.tile([1, 4 * SH], _F32, name="ps")
            # wait-consumer (see f32A): absorbs the xr-DMA wait so the first
            # real matmul only needs the weight-chunk DMA wait
            nc.tensor.matmul(
                ps[0:1, 0:1], xr_s[:, 0:1], xr_s[:, 0:1], start=True, stop=True
            )

            for c in range(n_chunks):
                w = wpool.tile(
                    [128, spd * 4 * SH], _F16, name=f"w{c}", tag="w"
                )
                src = w4[c * spd * 128 : (c + 1) * spd * 128, :].rearrange(
                    "(s p) n -> p s n", p=128
                )
                nc.sync.dma_start(
                    out=w.rearrange("p (s n) -> p s n", s=spd), in_=src
                )
                for s in range(spd):
                    kc = c * spd + s
                    for g in range(4):
                        nc.tensor.matmul(
                            ps[0:1, SH * g : SH * (g + 1)],
                            xr_s[:, kc : kc + 1],
                            w[:, 4 * SH * s + SH * g : 4 * SH * s + SH * (g + 1)],
                            start=(kc == 0),
                            stop=(kc == NKC - 1),
                        )

            pre = epool.tile([1, 4 * SH], _F32, name="pre")
            nc.vector.tensor_add(pre, ps[0:1, :], bias_s)
            # gate order: f(0:SH), i(SH:2SH), o(2SH:3SH), c(3SH:4SH)
            sig = epool.tile([1, 3 * SH], _F32, name="sig")
            nc.scalar.activation(sig, pre[:, 0 : 3 * SH], _AFT.Sigmoid)
            ztl = epool.tile([1, SH], _F32, name="ztl")
            nc.scalar.activation(ztl, pre[:, 3 * SH : 4 * SH], _AFT.Tanh)
            t1 = epool.tile([1, SH], _F32, name="t1")
            nc.vector.tensor_mul(t1, c0_s, sig[:, 0:SH])
            t2 = epool.tile([1, SH], _F32, name="t2")
            nc.vector.tensor_mul(t2, ztl, sig[:, SH : 2 * SH])
            cn = epool.tile([1, SH], _F32, name="cn")
            nc.vector.tensor_add(cn, t1, t2)
            tcn = epool.tile([1, SH], _F32, name="tcn")
            nc.scalar.activation(tcn, cn, _AFT.Tanh)
            hv = epool.tile([1, SH], _F32, name="hv")
            nc.vector.tensor_mul(hv, sig[:, 2 * SH : 3 * SH], tcn)
            nc.sync.dma_start(out=hout, in_=hv)
    return nc


def prep_f16B(x, h0, c0, Wf, bf, Wi, bi, Wc, bc, Wo, bo):
    concat = np.concatenate([h0[0], x[0]]).astype(np.float32)
    xr = np.ascontiguousarray(concat.reshape(NKC, 128).T).astype(np.float16)
    in_maps = []
    gw = [Wf, Wi, Wo, Wc]
    gb = [bf, bi, bo, bc]
    for ci in range(NCORES):
        lo = ci * SH
        w4 = np.ascontiguousarray(
            np.concatenate([W[:, lo : lo + SH] for W in gw], axis=1)
        ).astype(np.float16)
        bias = np.ascontiguousarray(
            np.concatenate([b[lo : lo + SH] for b in gb])
        ).astype(np.float32)[None, :]
        c0s = np.ascontiguousarray(c0[0, lo : lo + SH]).astype(np.float32)[
            None, :
        ]
        in_maps.append({"w4": w4, "xr": xr, "bias": bias, "c0s": c0s})
    return in_maps


def post_f16B(results):
    shards = [r["h"].reshape(SH) for r in results]
    return np.concatenate(shards)[None, :].astype(np.float32)




# chunk schedule for f16C: strips per DMA; small leading chunks cut the
# time-to-first-matmul, bigger ones amortize trigger cost in steady state
F16C_CHUNKS = [1, 1, 1, 1, 2, 2] + [4] * 14
F16C_WBUFS = 10


def build_f16C():
    """Like f16B but the weights arrive host-pre-transposed to the SBUF
    layout: wt[p, kc*2048 + j] = W4[128*kc + p, j]. Every weight DMA is a
    plain 2D slice with per-partition contiguous reads (few descriptors),
    and the chunk schedule starts with single strips so the PE gets work
    within a few microseconds."""
    nc = _new_bass()
    wt = nc.dram_tensor("wt", [128, NKC * 4 * SH], _F16, kind="ExternalInput").ap()
    xr = nc.dram_tensor("xr", [128, NKC], _F16, kind="ExternalInput").ap()
    bias = nc.dram_tensor("bias", [1, 4 * SH], _F32, kind="ExternalInput").ap()
    c0s = nc.dram_tensor("c0s", [1, SH], _F32, kind="ExternalInput").ap()
    hout = nc.dram_tensor("h", [1, SH], _F32, kind="ExternalOutput").ap()

    chunks = F16C_CHUNKS
    assert sum(chunks) == NKC
    with tile.TileContext(nc) as tc:
        with (
            tc.tile_pool(name="consts", bufs=1) as cpool,
            tc.tile_pool(name="wpool", bufs=F16C_WBUFS) as wpool,
            tc.tile_pool(name="ppool", bufs=1, space="PSUM") as ppool,
            tc.tile_pool(name="epool", bufs=1) as epool,
        ):
            xr_s = cpool.tile([128, NKC], _F16, name="xr_s")
            nc.sync.dma_start(out=xr_s, in_=xr)
            bias_s = cpool.tile([1, 4 * SH], _F32, name="bias_s")
            nc.sync.dma_start(out=bias_s, in_=bias)
            c0_s = cpool.tile([1, SH], _F32, name="c0_s")
            nc.sync.dma_start(out=c0_s, in_=c0s)

            ps = ppool.tile([1, 4 * SH], _F32, name="ps")
            nc.tensor.matmul(
                ps[0:1, 0:1], xr_s[:, 0:1], xr_s[:, 0:1], start=True, stop=True
            )

            kc = 0
            for ci, ns in enumerate(chunks):
                w = wpool.tile(
                    [128, ns * 4 * SH], _F16, name=f"w{ci}", tag="w"
                )
                base = kc * 4 * SH
                nc.sync.dma_start(
                    out=w, in_=wt[:, base : base + ns * 4 * SH]
                )
                for s in range(ns):
                    for g in range(4):
                        nc.tensor.matmul(
                            ps[0:1, SH * g : SH * (g + 1)],
                            xr_s[:, kc : kc + 1],
                            w[:, 4 * SH * s + SH * g : 4 * SH * s + SH * (g + 1)],
                            start=(kc == 0),
                            stop=(kc == NKC - 1),
                        )
                    kc += 1

            pre = epool.tile([1, 4 * SH], _F32, name="pre")
            nc.vector.tensor_add(pre, ps[0:1, :], bias_s)
            sig = epool.tile([1, 3 * SH], _F32, name="sig")
            nc.scalar.activation(sig, pre[:, 0 : 3 * SH], _AFT.Sigmoid)
            ztl = epool.tile([1, SH], _F32, name="ztl")
            nc.scalar.activation(ztl, pre[:, 3 * SH : 4 * SH], _AFT.Tanh)
            t1 = epool.tile([1, SH], _F32, name="t1")
            nc.vector.tensor_mul(t1, c0_s, sig[:, 0:SH])
            t2 = epool.tile([1, SH], _F32, name="t2")
            nc.vector.tensor_mul(t2, ztl, sig[:, SH : 2 * SH])
            cn = epool.tile([1, SH], _F32, name="cn")
            nc.vector.tensor_add(cn, t1, t2)
            tcn = epool.tile([1, SH], _F32, name="tcn")
            nc.scalar.activation(tcn, cn, _AFT.Tanh)
            hv = epool.tile([1, SH], _F32, name="hv")
            nc.vector.tensor_mul(hv, sig[:, 2 * SH : 3 * SH], tcn)
            nc.sync.dma_start(out=hout, in_=hv)
    return nc


def prep_f16C(x, h0, c0, Wf, bf, Wi, bi, Wc, bc, Wo, bo):
    concat = np.concatenate([h0[0], x[0]]).astype(np.float32)
    xr = np.ascontiguousarray(concat.reshape(NKC, 128).T).astype(np.float16)
    in_maps = []
    gw = [Wf, Wi, Wo, Wc]
    gb = [bf, bi, bo, bc]
    for ci in range(NCORES):
        lo = ci * SH
        w4 = np.concatenate(
            [W[:, lo : lo + SH] for W in gw], axis=1
        ).astype(np.float16)
        wt = np.ascontiguousarray(
            w4.reshape(NKC, 128, 4 * SH).transpose(1, 0, 2).reshape(128, -1)
        )
        bias = np.ascontiguousarray(
            np.concatenate([b[lo : lo + SH] for b in gb])
        ).astype(np.float32)[None, :]
        c0s = np.ascontiguousarray(c0[0, lo : lo + SH]).astype(np.float32)[
            None, :
        ]
        in_maps.append({"wt": wt, "xr": xr, "bias": bias, "c0s": c0s})
    return in_maps



_VARIANTS = {
    "f32A": (build_f32A, prep_f32A, post_f32A),
    "f16B": (build_f16B, prep_f16B, post_f16B),
    "f16C": (build_f16C, prep_f16C, post_f16B),
}


def run_variant(variant, inputs, trace=False, **spmd_kwargs):
    build, prep, post = _VARIANTS[variant]
    nc = build()
    # post-scheduling pass for walrus's one-wait-per-instruction limit
    # (CoreSim can't execute the injected nops, so this is HW-path only)
    _split_multiwaits(nc)
    in_maps = prep(**inputs)
    res = run_bass_kernel_spmd(
        nc, in_maps, list(range(NCORES)), trace=trace, **spmd_kwargs
    )
    return post(res.results), res


def kernel(**inputs):
    out, _ = run_variant(VARIANT, inputs)
    return out


# revision 17
# speedup vs baseline: 1.2009x; 1.1631x over previous
"""Trainium2 Bass kernel for a batch-1 LSTM cell (D=4096).

Math (per reference):
    concat = [h0, x]                       # [1, 8192]
    z  = tanh(concat @ Wc + bc)
    zf = sigmoid(concat @ Wf + bf)
    zi = sigmoid(concat @ Wi + bi)
    zo = sigmoid(concat @ Wo + bo)
    c  = c0 * zf + z * zi
    h  = zo * tanh(c)                      # [1, 4096]  (returned)

Sharding: tensor-parallel over the gate output dim. Core ci owns output
columns [ci*512, (ci+1)*512) of all four gates: it streams its
[8192, 4*512] weight block from HBM, computes the four gate slices, and
the elementwise state update for its 512 lanes. Outputs are concatenated
on the host.

The dominant cost is streaming the weights (512 MiB fp32 total, 64 MiB
per core). The matmul is a matvec, run with the weight tile as the
stationary operand (fp32 moving operand would run at 1/4 rate) and the
activation chunk [128, 1] as the moving operand, accumulating over the
64 K-chunks into PSUM.

Variants:
  f32A: fp32 weights, weights-stationary matvec. Two passes (gates f,i
        then o,c) so each of the 8 concurrent PSUM accumulation series
        owns a whole bank — no cross-series has_written interference.
  f16B: fp16 weights (host-downcast), activation-chunk stationary,
        weight strips moving at 1 col/cycle. Half the HBM bytes.
"""

import numpy as np

import concourse.bass as bass
import concourse.mybir as mybir
import concourse.tile as tile
from concourse.bass_utils import run_bass_kernel_spmd

D = 4096
K = 2 * D          # concat length, 8192
NCORES = 8
SH = D // NCORES   # 512 output columns per core per gate
NKC = K // 128     # 64 K-chunks of 128

VARIANT = "f16B"
STRIPS_PER_DMA = 4
W_BUFS = 8

_F32 = mybir.dt.float32
_F16 = mybir.dt.float16
_AFT = mybir.ActivationFunctionType


def _new_bass():
    return bass.Bass(
        trn_type="TRN2",
        target_bir_lowering=False,
        debug=False,
        num_devices=NCORES,
    )


# Instruction types walrus lowers with multi-wait support (sequencer loops).
_MULTIWAIT_OK = ("InstAllEngineBarrier", "InstNoOp", "InstISA")


def _split_multiwaits(nc):
    """Walrus encodes at most one sync-wait on compute/DMA instructions on
    this toolchain (static-DMA DIRECT2D / S3_LW structs). Tile's semaphore
    assigner sometimes emits 2+. Hoist the extras onto same-engine no-ops
    inserted immediately before the instruction — the sequencer executes the
    nop waits first, which is semantically identical."""
    n = 0
    for fn in nc.m.functions:
        for blk in fn.blocks:
            insts = blk.instructions
            i = 0
            while i < len(insts):
                inst = insts[i]
                si = inst.sync_info
                waits = list(si.on_wait) if si and si.on_wait else []
                if (
                    len(waits) > 1
                    and type(inst).__name__ not in _MULTIWAIT_OK
                ):
                    for w in waits[:-1]:
                        nop = mybir.InstNoOp(
                            name=f"I-waitnop{n}",
                            engine=inst.engine,
                            ins=[],
                            outs=[],
                            sync_info=mybir.SyncInfo(
                                on_wait=[w], on_update=[]
                            ),
                        )
                        insts.insert(i, nop)
                        n += 1
                        i += 1
                    inst.sync_info = mybir.SyncInfo(
                        on_wait=[waits[-1]], on_update=list(si.on_update)
                    )
                i += 1
    return n


def build_f32A():
    """fp32, weights stationary. Two gate-pair passes, 8 PSUM banks each.

    Weight inputs: wa = [K, 2*SH] (gates f,i), wb = [K, 2*SH] (gates o,c).
    xr   = [128, NKC]  xr[p, kc] = concat[128*kc + p]
    bias = [128, 16]   bias[p, 4*g + t] = b_g[shard + 128*t + p], g in f,i,o,c
    c0s  = [128, 4]    c0s[p, t] = c0[shard + 128*t + p]
    out h = [128, 4]   h[p, t] = h_full[shard + 128*t + p]
    """
    nc = _new_bass()
    wa = nc.dram_tensor("wa", [K, 2 * SH], _F32, kind="ExternalInput").ap()
    wb = nc.dram_tensor("wb", [K, 2 * SH], _F32, kind="ExternalInput").ap()
    xr = nc.dram_tensor("xr", [128, NKC], _F32, kind="ExternalInput").ap()
    bias = nc.dram_tensor("bias", [128, 16], _F32, kind="ExternalInput").ap()
    c0s = nc.dram_tensor("c0s", [128, 4], _F32, kind="ExternalInput").ap()
    hout = nc.dram_tensor("h", [128, 4], _F32, kind="ExternalOutput").ap()

    spd = STRIPS_PER_DMA
    n_chunks = NKC // spd
    with tile.TileContext(nc) as tc:
        with (
            tc.tile_pool(name="consts", bufs=1) as cpool,
            tc.tile_pool(name="wpool", bufs=W_BUFS) as wpool,
            tc.tile_pool(name="ppool", bufs=1, space="PSUM") as ppool,
            tc.tile_pool(name="epool", bufs=1) as epool,
        ):
            xr_s = cpool.tile([128, NKC], _F32, name="xr_s")
            nc.sync.dma_start(out=xr_s, in_=xr)
            bias_s = cpool.tile([128, 16], _F32, name="bias_s")
            nc.sync.dma_start(out=bias_s, in_=bias)
            c0_s = cpool.tile([128, 4], _F32, name="c0_s")
            nc.sync.dma_start(out=c0_s, in_=c0s)

            # pre-activations (bias added), laid out [128, 4*g + t]
            pre = epool.tile([128, 16], _F32, name="pre")

            # 8 accumulator banks, shared by both gate-pair phases (reusing
            # the same tiles avoids pool slot-reuse semaphores, which would
            # pile >1 wait onto a matmul — walrus allows exactly one).
            ps = []
            for i in range(8):
                ps.append(ppool.tile([128, 1], _F32, name=f"ps{i}"))

            for ph, wsrc in ((0, wa), (1, wb)):
                # Wait-consumer: walrus matmuls have one sync-wait slot, but
                # the first matmul of a phase would need two (xr-DMA or
                # psum-evacuation wait plus the weight-chunk DMA wait). Run a
                # throwaway complete accumulation group on ps[0] that
                # consumes the non-DMA wait; the real series then re-starts
                # the bank and overwrites.
                nc.tensor.matmul(
                    ps[0][0:1, 0:1],
                    xr_s[:, 0:1],
                    xr_s[:, 0:1],
                    start=True,
                    stop=True,
                )
                for c in range(n_chunks):
                    w = wpool.tile(
                        [128, spd * 2 * SH], _F32, name=f"w{ph}_{c}", tag="w"
                    )
                    src = wsrc[c * spd * 128 : (c + 1) * spd * 128, :].rearrange(
                        "(s p) n -> p s n", p=128
                    )
                    nc.sync.dma_start(
                        out=w.rearrange("p (s n) -> p s n", s=spd), in_=src
                    )
                    for s in range(spd):
                        kc = c * spd + s
                        for gg in range(2):  # gate within pair
                            for t in range(4):
                                nc.tensor.matmul(
                                    ps[4 * gg + t][:, 0:1],
                                    w[
                                        :,
                                        2 * SH * s
                                        + SH * gg
                                        + 128 * t : 2 * SH * s
                                        + SH * gg
                                        + 128 * t
                                        + 128,
                                    ],
                                    xr_s[:, kc : kc + 1],
                                    start=(kc == 0),
                                    stop=(kc == NKC - 1),
                                )
                # evacuate with bias add: gates 2*ph + gg
                for gg in range(2):
                    g = 2 * ph + gg
                    for t in range(4):
                        nc.vector.tensor_add(
                            pre[:, 4 * g + t : 4 * g + t + 1],
                            ps[4 * gg + t][:, 0:1],
                            bias_s[:, 4 * g + t : 4 * g + t + 1],
                        )

            # gate order: f(0:4), i(4:8), o(8:12), c(12:16)
            sig = epool.tile([128, 12], _F32, name="sig")
            nc.scalar.activation(sig, pre[:, 0:12], _AFT.Sigmoid)
            ztl = epool.tile([128, 4], _F32, name="ztl")
            nc.scalar.activation(ztl, pre[:, 12:16], _AFT.Tanh)
            t1 = epool.tile([128, 4], _F32, name="t1")
            nc.vector.tensor_mul(t1, c0_s, sig[:, 0:4])
            t2 = epool.tile([128, 4], _F32, name="t2")
            nc.vector.tensor_mul(t2, ztl, sig[:, 4:8])
            cn = epool.tile([128, 4], _F32, name="cn")
            nc.vector.tensor_add(cn, t1, t2)
            tcn = epool.tile([128, 4], _F32, name="tcn")
            nc.scalar.activation(tcn, cn, _AFT.Tanh)
            hv = epool.tile([128, 4], _F32, name="hv")
            nc.vector.tensor_mul(hv, sig[:, 8:12], tcn)
            nc.sync.dma_start(out=hout, in_=hv)
    return nc


def prep_f32A(x, h0, c0, Wf, bf, Wi, bi, Wc, bc, Wo, bo):
    concat = np.concatenate([h0[0], x[0]]).astype(np.float32)
    xr = np.ascontiguousarray(concat.reshape(NKC, 128).T)
    in_maps = []
    gw = [Wf, Wi, Wo, Wc]
    gb = [bf, bi, bo, bc]
    for ci in range(NCORES):
        lo = ci * SH
        wa = np.ascontiguousarray(
            np.concatenate([W[:, lo : lo + SH] for W in gw[:2]], axis=1)
        )
        wb = np.ascontiguousarray(
            np.concatenate([W[:, lo : lo + SH] for W in gw[2:]], axis=1)
        )
        bias = np.ascontiguousarray(
            np.concatenate(
                [b[lo : lo + SH].reshape(4, 128).T for b in gb], axis=1
            )
        )
        c0s = np.ascontiguousarray(c0[0, lo : lo + SH].reshape(4, 128).T)
        in_maps.append(
            {"wa": wa, "wb": wb, "xr": xr, "bias": bias, "c0s": c0s}
        )
    return in_maps


def post_f32A(results):
    shards = [r["h"].T.reshape(SH) for r in results]
    return np.concatenate(shards)[None, :].astype(np.float32)


def build_f16B():
    """fp16 weights moving, activation chunk stationary. Single pass.

    w4  = [K, 4*SH] fp16, gate order f,i,o,c along columns
    xr  = [128, NKC] fp16 (stationary chunks)
    bias = [1, 4*SH] fp32, c0s = [1, SH] fp32, out h = [1, SH] fp32
    """
    nc = _new_bass()
    w4 = nc.dram_tensor("w4", [K, 4 * SH], _F16, kind="ExternalInput").ap()
    xr = nc.dram_tensor("xr", [128, NKC], _F16, kind="ExternalInput").ap()
    bias = nc.dram_tensor("bias", [1, 4 * SH], _F32, kind="ExternalInput").ap()
    c0s = nc.dram_tensor("c0s", [1, SH], _F32, kind="ExternalInput").ap()
    hout = nc.dram_tensor("h", [1, SH], _F32, kind="ExternalOutput").ap()

    spd = STRIPS_PER_DMA
    n_chunks = NKC // spd
    with tile.TileContext(nc) as tc:
        with (
            tc.tile_pool(name="consts", bufs=1) as cpool,
            tc.tile_pool(name="wpool", bufs=W_BUFS) as wpool,
            tc.tile_pool(name="ppool", bufs=1, space="PSUM") as ppool,
            tc.tile_pool(name="epool", bufs=1) as epool,
        ):
            xr_s = cpool.tile([128, NKC], _F16, name="xr_s")
            nc.sync.dma_start(out=xr_s, in_=xr)
            bias_s = cpool.tile([1, 4 * SH], _F32, name="bias_s")
            nc.sync.dma_start(out=bias_s, in_=bias)
            c0_s = cpool.tile([1, SH], _F32, name="c0_s")
            nc.sync.dma_start(out=c0_s, in_=c0s)

            # one accumulator bank per gate, [1, 512] each on partition 0
            ps = ppool.tile([1, 4 * SH], _F32, name="ps")
            # wait-consumer (see f32A): absorbs the xr-DMA wait so the first
            # real matmul only needs the weight-chunk DMA wait
            nc.tensor.matmul(
                ps[0:1, 0:1], xr_s[:, 0:1], xr_s[:, 0:1], start=True, stop=True
            )

            for c in range(n_chunks):
                w = wpool.tile(
                    [128, spd * 4 * SH], _F16, name=f"w{c}", tag="w"
                )
                src = w4[c * spd * 128 : (c + 1) * spd * 128, :].rearrange(
                    "(s p) n -> p s n", p=128
                )
                nc.sync.dma_start(
                    out=w.rearrange("p (s n) -> p s n", s=spd), in_=src
                )
                for s in range(spd):
                    kc = c * spd + s
                    for g in range(4):
                        nc.tensor.matmul(
                            ps[0:1, SH * g : SH * (g + 1)],
                            xr_s[:, kc : kc + 1],
                            w[:, 4 * SH * s + SH * g : 4 * SH * s + SH * (g + 1)],
                            start=(kc == 0),
                            stop=(kc == NKC - 1),
                        )

            pre = epool.tile([1, 4 * SH], _F32, name="pre")
            nc.vector.tensor_add(pre, ps[0:1, :], bias_s)
            # gate order: f(0:SH), i(SH:2SH), o(2SH:3SH), c(3SH:4SH)
            sig = epool.tile([1, 3 * SH], _F32, name="sig")
            nc.scalar.activation(sig, pre[:, 0 : 3 * SH], _AFT.Sigmoid)
            ztl = epool.tile([1, SH], _F32, name="ztl")
            nc.scalar.activation(ztl, pre[:, 3 * SH : 4 * SH], _AFT.Tanh)
            t1 = epool.tile([1, SH], _F32, name="t1")
            nc.vector.tensor_mul(t1, c0_s, sig[:, 0:SH])
            t2 = epool.tile([1, SH], _F32, name="t2")
            nc.vector.tensor_mul(t2, ztl, sig[:, SH : 2 * SH])
            cn = epool.tile([1, SH], _F32, name="cn")
            nc.vector.tensor_add(cn, t1, t2)
            tcn = epool.tile([1, SH], _F32, name="tcn")
            nc.scalar.activation(tcn, cn, _AFT.Tanh)
            hv = epool.tile([1, SH], _F32, name="hv")
            nc.vector.tensor_mul(hv, sig[:, 2 * SH : 3 * SH], tcn)
            nc.sync.dma_start(out=hout, in_=hv)
    return nc


def prep_f16B(x, h0, c0, Wf, bf, Wi, bi, Wc, bc, Wo, bo):
    concat = np.concatenate([h0[0], x[0]]).astype(np.float32)
    xr = np.ascontiguousarray(concat.reshape(NKC, 128).T).astype(np.float16)
    in_maps = []
    gw = [Wf, Wi, Wo, Wc]
    gb = [bf, bi, bo, bc]
    for ci in range(NCORES):
        lo = ci * SH
        w4 = np.ascontiguousarray(
            np.concatenate([W[:, lo : lo + SH] for W in gw], axis=1)
        ).astype(np.float16)
        bias = np.ascontiguousarray(
            np.concatenate([b[lo : lo + SH] for b in gb])
        ).astype(np.float32)[None, :]
        c0s = np.ascontiguousarray(c0[0, lo : lo + SH]).astype(np.float32)[
            None, :
        ]
        in_maps.append({"w4": w4, "xr": xr, "bias": bias, "c0s": c0s})
    return in_maps


def post_f16B(results):
    shards = [r["h"].reshape(SH) for r in results]
    return np.concatenate(shards)[None, :].astype(np.float32)




# chunk schedule for f16C: strips per DMA; small leading chunks cut the
# time-to-first-matmul, bigger ones amortize trigger cost in steady state
F16C_CHUNKS = [1, 1, 1, 1, 2, 2] + [4] * 13 + [2, 1, 1]
F16C_WBUFS = 10
F16C_WARMUP_MMS = 14


def build_f16C():
    """Like f16B but the weights arrive host-pre-transposed to the SBUF
    layout: wt[p, kc*2048 + j] = W4[128*kc + p, j]. Every weight DMA is a
    plain 2D slice with per-partition contiguous reads (few descriptors),
    and the chunk schedule starts with single strips so the PE gets work
    within a few microseconds."""
    nc = _new_bass()
    wt = nc.dram_tensor("wt", [128, NKC * 4 * SH], _F16, kind="ExternalInput").ap()
    xr = nc.dram_tensor("xr", [128, NKC], _F16, kind="ExternalInput").ap()
    bias = nc.dram_tensor("bias", [1, 4 * SH], _F32, kind="ExternalInput").ap()
    c0s = nc.dram_tensor("c0s", [1, SH], _F32, kind="ExternalInput").ap()
    hout = nc.dram_tensor("h", [1, SH], _F32, kind="ExternalOutput").ap()

    chunks = F16C_CHUNKS
    assert sum(chunks) == NKC
    with tile.TileContext(nc) as tc:
        with (
            tc.tile_pool(name="consts", bufs=1) as cpool,
            tc.tile_pool(name="wpool", bufs=F16C_WBUFS) as wpool,
            tc.tile_pool(name="ppool", bufs=1, space="PSUM") as ppool,
            tc.tile_pool(name="epool", bufs=1) as epool,
        ):
            xr_s = cpool.tile([128, NKC], _F16, name="xr_s")
            nc.sync.dma_start(out=xr_s, in_=xr)
            bias_s = cpool.tile([1, 4 * SH], _F32, name="bias_s")
            nc.sync.dma_start(out=bias_s, in_=bias)
            c0_s = cpool.tile([1, SH], _F32, name="c0_s")
            nc.sync.dma_start(out=c0_s, in_=c0s)

            ps = ppool.tile([1, 4 * SH], _F32, name="ps")

            # PE warm-up: ~6us of dummy matmuls with no DMA dependency so
            # the HAM clock-gate reaches 8/8 before the real stream, and the
            # PE never falls behind the DMA pace (cold MMs are 2x slower).
            zmov = cpool.tile([128, SH], _F16, name="zmov")
            nc.vector.memset(zmov, 0.0)
            dps = ppool.tile([1, SH], _F32, name="dps")
            for wu in range(F16C_WARMUP_MMS):
                nc.tensor.matmul(
                    dps[0:1, :], zmov[:, 0:1], zmov, start=True, stop=True
                )
            # preload the sigmoid/tanh ACT LUTs during the stream instead of
            # paying the table-load latency in the kernel tail
            zact = epool.tile([1, 2], _F32, name="zact")
            nc.vector.memset(zact, 0.0)
            nc.scalar.activation(zact[:, 0:1], zact[:, 0:1], _AFT.Sigmoid)
            nc.scalar.activation(zact[:, 1:2], zact[:, 1:2], _AFT.Tanh)

            kc = 0
            for ci, ns in enumerate(chunks):
                w = wpool.tile(
                    [128, ns * 4 * SH], _F16, name=f"w{ci}", tag="w"
                )
                base = kc * 4 * SH
                nc.sync.dma_start(
                    out=w, in_=wt[:, base : base + ns * 4 * SH]
                )
                for s in range(ns):
                    for g in range(4):
                        nc.tensor.matmul(
                            ps[0:1, SH * g : SH * (g + 1)],
                            xr_s[:, kc : kc + 1],
                            w[:, 4 * SH * s + SH * g : 4 * SH * s + SH * (g + 1)],
                            start=(kc == 0),
                            stop=(kc == NKC - 1),
                        )
                    kc += 1

            pre = epool.tile([1, 4 * SH], _F32, name="pre")
            nc.vector.tensor_add(pre, ps[0:1, :], bias_s)
            sig = epool.tile([1, 3 * SH], _F32, name="sig")
            nc.scalar.activation(sig, pre[:, 0 : 3 * SH], _AFT.Sigmoid)
            ztl = epool.tile([1, SH], _F32, name="ztl")
            nc.scalar.activation(ztl, pre[:, 3 * SH : 4 * SH], _AFT.Tanh)
            t1 = epool.tile([1, SH], _F32, name="t1")
            nc.vector.tensor_mul(t1, c0_s, sig[:, 0:SH])
            t2 = epool.tile([1, SH], _F32, name="t2")
            nc.vector.tensor_mul(t2, ztl, sig[:, SH : 2 * SH])
            cn = epool.tile([1, SH], _F32, name="cn")
            nc.vector.tensor_add(cn, t1, t2)
            tcn = epool.tile([1, SH], _F32, name="tcn")
            nc.scalar.activation(tcn, cn, _AFT.Tanh)
            hv = epool.tile([1, SH], _F32, name="hv")
            nc.vector.tensor_mul(hv, sig[:, 2 * SH : 3 * SH], tcn)
            nc.sync.dma_start(out=hout, in_=hv)
    return nc


def prep_f16C(x, h0, c0, Wf, bf, Wi, bi, Wc, bc, Wo, bo):
    concat = np.concatenate([h0[0], x[0]]).astype(np.float32)
    xr = np.ascontiguousarray(concat.reshape(NKC, 128).T).astype(np.float16)
    in_maps = []
    gw = [Wf, Wi, Wo, Wc]
    gb = [bf, bi, bo, bc]
    for ci in range(NCORES):
        lo = ci * SH
        w4 = np.concatenate(
            [W[:, lo : lo + SH] for W in gw], axis=1
        ).astype(np.float16)
        wt = np.ascontiguousarray(
            w4.reshape(NKC, 128, 4 * SH).transpose(1, 0, 2).reshape(128, -1)
        )
        bias = np.ascontiguousarray(
            np.concatenate([b[lo : lo + SH] for b in gb])
        ).astype(np.float32)[None, :]
        c0s = np.ascontiguousarray(c0[0, lo : lo + SH]).astype(np.float32)[
            None, :
        ]
        in_maps.append({"wt": wt, "xr": xr, "bias": bias, "c0s": c0s})
    return in_maps



_VARIANTS = {
    "f32A": (build_f32A, prep_f32A, post_f32A),
    "f16B": (build_f16B, prep_f16B, post_f16B),
    "f16C": (build_f16C, prep_f16C, post_f16B),
}


def run_variant(variant, inputs, trace=False, **spmd_kwargs):
    build, prep, post = _VARIANTS[variant]
    nc = build()
    # post-scheduling pass for walrus's one-wait-per-instruction limit
    # (CoreSim can't execute the injected nops, so this is HW-path only)
    _split_multiwaits(nc)
    in_maps = prep(**inputs)
    res = run_bass_kernel_spmd(
        nc, in_maps, list(range(NCORES)), trace=trace, **spmd_kwargs
    )
    return post(res.results), res


def kernel(**inputs):
    out, _ = run_variant(VARIANT, inputs)
    return out
